# revision 13
# baseline (speedup 1.0000x reference)
"""GCN encoder (2-layer) on 8 Trainium2 NeuronCores.

Sharding: nodes split into 8 contiguous ranges (graph/data parallel).
Each core:
  1. GEMM  g1 = dinv * (x_shard @ W1)          (dinv folded into table rows)
  2. AllGather g1 shards -> full table G1
  3. For each 128-dst window: dma_gather source rows of G1, build one-hot
     selection matrices S on DVE (iota == dst_local), scatter-add via
     PE matmul  psum += S.T @ rows ; h1 = relu(dinv * psum + b1)
  4. GEMM  g2 = dinv * (h1 @ W2), AllGather -> G2, same aggregation,
     out = dinv * psum + b2
Host side only does sharding/index prep (CSR-style sort, int16 index
packing) and the final unshard.
"""

import os
import numpy as np

import concourse.bass as bass
import concourse.bacc as bacc
import concourse.mybir as mybir
from concourse.tile import TileContext
from concourse.tile_rust import add_dep_helper
from concourse.bass_utils import run_bass_kernel_spmd

F32 = mybir.dt.float32
I16 = mybir.dt.int16
M = 8           # cores
P = 128         # partitions
SPLIT = 32768   # int16 index limit for dma_gather


def _build_nc(IN_C, HID, OUT, NW, CA, CB, has_b1, has_b2):
    """Build the SPMD Bass program. CA/CB: per-window tile capacities."""
    STAGE = float(os.environ.get("GCN_STAGE", "4"))
    NPAD = NW * P
    KT1 = IN_C // P
    colsA = max(8, int(np.sum(CA)) * 8)   # idx cols (16 idx per col)
    colsB = max(8, int(np.sum(CB)) * 8)
    TTOT = max(1, int(np.sum(CA) + np.sum(CB)))

    nc = bacc.Bacc("TRN2", target_bir_lowering=False, debug=False, num_devices=M)
    xT = nc.dram_tensor("xT", [IN_C, NPAD], F32, kind="ExternalInput")
    W1 = nc.dram_tensor("W1", [IN_C, HID], F32, kind="ExternalInput")
    W2 = nc.dram_tensor("W2", [HID, OUT], F32, kind="ExternalInput")
    b1 = nc.dram_tensor("b1", [P, HID], F32, kind="ExternalInput")
    b2 = nc.dram_tensor("b2", [P, OUT], F32, kind="ExternalInput")
    deg = nc.dram_tensor("deg", [P, NW], F32, kind="ExternalInput")
    idxA = nc.dram_tensor("idxA", [P, colsA], I16, kind="ExternalInput")
    idxB = nc.dram_tensor("idxB", [P, colsB], I16, kind="ExternalInput")
    dloc = nc.dram_tensor("dloc", [P, TTOT], F32, kind="ExternalInput")
    iota = nc.dram_tensor("iota", [P, P], F32, kind="ExternalInput")
    ident = nc.dram_tensor("ident", [P, P], F32, kind="ExternalInput")
    out = nc.dram_tensor("out", [NPAD, OUT], F32, kind="ExternalOutput")
    G1 = nc.dram_tensor("G1", [M * NPAD, HID], F32, addr_space="Shared")
    G2 = nc.dram_tensor("G2", [M * NPAD, OUT], F32, addr_space="Shared")

    xT_v = xT.rearrange("(kt k) n -> kt k n", k=P)
    W1_v = W1.rearrange("(kt k) h -> k kt h", k=P)

    with TileContext(nc) as tc:
        with (
            tc.tile_pool(name="const", bufs=1) as const,
            tc.tile_pool(name="big", bufs=1) as big,
            tc.tile_pool(name="work", bufs=4) as work,
            tc.tile_pool(name="spool", bufs=4) as spool,
            tc.tile_pool(name="gpool", bufs=2) as gpool,
            tc.tile_pool(name="psum", bufs=2, space="PSUM") as psum,
            tc.tile_pool(name="dram", bufs=1, space="DRAM") as dram,
        ):
            g1loc = dram.tile([NPAD, HID], F32)
            g2loc = dram.tile([NPAD, OUT], F32)
            # ---- constants ----
            w1_sb = const.tile([P, KT1, HID], F32)
            nc.sync.dma_start(out=w1_sb[:], in_=W1_v[:])
            w2_sb = const.tile([P, OUT], F32)
            nc.sync.dma_start(out=w2_sb[:], in_=W2[:])
            iota_sb = const.tile([P, P], F32)
            nc.sync.dma_start(out=iota_sb[:], in_=iota[:])
            id_sb = const.tile([P, P], F32)
            nc.sync.dma_start(out=id_sb[:], in_=ident[:])
            deg_sb = const.tile([P, NW], F32)
            nc.sync.dma_start(out=deg_sb[:], in_=deg[:])
            dloc_sb = big.tile([P, TTOT], F32)
            nc.sync.dma_start(out=dloc_sb[:], in_=dloc[:])
            b1_sb = const.tile([P, HID], F32)
            b2_sb = const.tile([P, OUT], F32)
            if has_b1:
                nc.sync.dma_start(out=b1_sb[:], in_=b1[:])
            if has_b2:
                nc.sync.dma_start(out=b2_sb[:], in_=b2[:])

            dinv_sb = const.tile([P, NW], F32)
            nc.vector.reciprocal(out=dinv_sb[:], in_=deg_sb[:])
            nc.scalar.sqrt(out=dinv_sb[:], in_=dinv_sb[:])

            g1_sb = big.tile([P, NW, HID], F32)
            h1_sb = big.tile([P, NW, HID], F32)
            g2_sb = big.tile([P, NW, OUT], F32)
            out_sb = big.tile([P, NW, OUT], F32)

            # ---- GEMM1: g1 = dinv * (x @ W1) ----
            for w in range(NW):
                ps = psum.tile([P, HID], F32, tag="g1ps")
                for k in range(KT1):
                    xt = work.tile([P, P], F32, tag="xt")
                    nc.sync.dma_start(
                        out=xt[:], in_=xT_v[k, :, w * P:(w + 1) * P])
                    nc.tensor.matmul(
                        ps[:], lhsT=xt[:], rhs=w1_sb[:, k, :],
                        start=(k == 0), stop=(k == KT1 - 1))
                nc.vector.tensor_scalar(
                    out=g1_sb[:, w, :], in0=ps[:],
                    scalar1=dinv_sb[:, w:w + 1], scalar2=None,
                    op0=mybir.AluOpType.mult)
            nc.gpsimd.dma_start(
                out=g1loc[:].rearrange("(w p) h -> p w h", p=P), in_=g1_sb[:])
            cc1 = nc.gpsimd.collective_compute(
                "AllGather", mybir.AluOpType.bypass,
                replica_groups=[list(range(M))],
                ins=[g1loc.opt()], outs=[G1[:]])

            if STAGE == 1:
                nc.sync.dma_start(
                    out=out.rearrange("(w p) h -> p w h", p=P)[:],
                    in_=g1_sb[:, :, :OUT])
            # ---- aggregation over edges ----
            def aggregate(G_ap, elem, dst_view, relu, bias_sb, cc):
                offA = offB = tcol = 0
                for w in range(NW):
                    ps = psum.tile([P, HID], F32, tag="agg")
                    nmm = CA[w] + CB[w]
                    mm = 0
                    CHUNK = 6   # <=768 idx per dma_gather (HW ring limit)
                    for g in range(2):
                        C = (CA, CB)[g][w]
                        if C == 0:
                            continue
                        off = (offA, offB)[g]
                        in_ap = G_ap[SPLIT:, :] if g == 1 else G_ap[:]
                        for c0 in range(0, C, CHUNK):
                            Cc = min(CHUNK, C - c0)
                            nidx = Cc * P
                            gb = gpool.tile([P, Cc, elem], F32, tag=f"gb{g}")
                            it = spool.tile([P, Cc * 8], I16, tag=f"idx{g}")
                            nc.sync.dma_start(
                                out=it[:],
                                in_=(idxA, idxB)[g][
                                    :, off + c0 * 8:off + (c0 + Cc) * 8])
                            gi = nc.gpsimd.dma_gather(
                                out_ap=gb[:], in_ap=in_ap,
                                idxs_ap=it[:],
                                num_idxs=nidx, num_idxs_reg=nidx,
                                elem_size=elem)
                            add_dep_helper(gi.ins, cc.ins, sync=True,
                                           reason="gather after allgather")
                            for t in range(Cc):
                                S = spool.tile([P, P], F32, tag="S")
                                nc.vector.tensor_scalar(
                                    out=S[:], in0=iota_sb[:],
                                    scalar1=dloc_sb[:, tcol:tcol + 1],
                                    scalar2=None,
                                    op0=mybir.AluOpType.is_equal)
                                nc.tensor.matmul(
                                    ps[:, :elem], lhsT=S[:], rhs=gb[:, t, :],
                                    start=(mm == 0), stop=(mm == nmm - 1))
                                mm += 1
                                tcol += 1
                        if g == 0:
                            offA += C * 8
                        else:
                            offB += C * 8
                    if bias_sb is None:
                        if relu:
                            nc.vector.tensor_scalar(
                                out=dst_view[:, w, :], in0=ps[:, :elem],
                                scalar1=dinv_sb[:, w:w + 1], scalar2=0.0,
                                op0=mybir.AluOpType.mult,
                                op1=mybir.AluOpType.max)
                        else:
                            nc.vector.tensor_scalar(
                                out=dst_view[:, w, :], in0=ps[:, :elem],
                                scalar1=dinv_sb[:, w:w + 1], scalar2=None,
                                op0=mybir.AluOpType.mult)
                    else:
                        tmp = work.tile([P, elem], F32, tag="evtmp")
                        nc.vector.tensor_scalar(
                            out=tmp[:], in0=ps[:, :elem],
                            scalar1=dinv_sb[:, w:w + 1], scalar2=None,
                            op0=mybir.AluOpType.mult)
                        if relu:
                            nc.vector.tensor_tensor(
                                out=tmp[:], in0=tmp[:], in1=bias_sb[:],
                                op=mybir.AluOpType.add)
                            nc.vector.tensor_scalar(
                                out=dst_view[:, w, :], in0=tmp[:],
                                scalar1=0.0, scalar2=None,
                                op0=mybir.AluOpType.max)
                        else:
                            nc.vector.tensor_tensor(
                                out=dst_view[:, w, :], in0=tmp[:],
                                in1=bias_sb[:], op=mybir.AluOpType.add)

            if STAGE == 1.5:
                tg = big.tile([P, NW, OUT], F32)
                nc.sync.dma_start(
                    out=tg[:], in_=G1.rearrange(
                        "(w p) h -> p w h", p=P)[:, :NW, :OUT])
                nc.sync.dma_start(
                    out=out.rearrange("(w p) h -> p w h", p=P)[:], in_=tg[:])
            if STAGE == 1.75:
                C0 = CA[0]
                gb0 = gpool.tile([P, C0, HID], F32, tag="gb0")
                it0 = spool.tile([P, C0 * 8], I16, tag="idx0")
                nc.sync.dma_start(out=it0[:], in_=idxA[:, 0:C0 * 8])
                nc.gpsimd.dma_gather(
                    out_ap=gb0[:], in_ap=G1[:],
                    idxs_ap=it0[:],
                    num_idxs=C0 * P, num_idxs_reg=C0 * P, elem_size=HID)
                nc.sync.dma_start(
                    out=out.rearrange("(w p) h -> p w h", p=P)[:, :1, :],
                    in_=gb0[:, :1, :OUT])
            if STAGE >= 2:
                aggregate(G1[:], HID, h1_sb, True, b1_sb if has_b1 else None, cc1)
            if STAGE == 2:
                nc.sync.dma_start(
                    out=out.rearrange("(w p) h -> p w h", p=P)[:],
                    in_=h1_sb[:, :, :OUT])

            # ---- GEMM2: g2 = dinv * (h1 @ W2) ----
            for w in (range(NW) if STAGE >= 3 else []):
                pst = psum.tile([P, P], F32, tag="tr")
                nc.tensor.transpose(
                    out=pst[:], in_=h1_sb[:, w, :], identity=id_sb[:])
                h1t = work.tile([P, P], F32, tag="h1t")
                nc.vector.tensor_copy(out=h1t[:], in_=pst[:])
                ps2 = psum.tile([P, OUT], F32, tag="mm2")
                nc.tensor.matmul(
                    ps2[:], lhsT=h1t[:], rhs=w2_sb[:], start=True, stop=True)
                nc.vector.tensor_scalar(
                    out=g2_sb[:, w, :], in0=ps2[:],
                    scalar1=dinv_sb[:, w:w + 1], scalar2=None,
                    op0=mybir.AluOpType.mult)
            if STAGE >= 3:
                nc.gpsimd.dma_start(
                    out=g2loc[:].rearrange("(w p) h -> p w h", p=P), in_=g2_sb[:])
                cc2 = nc.gpsimd.collective_compute(
                    "AllGather", mybir.AluOpType.bypass,
                    replica_groups=[list(range(M))],
                    ins=[g2loc.opt()], outs=[G2[:]])
            if STAGE == 3:
                nc.sync.dma_start(
                    out=out.rearrange("(w p) h -> p w h", p=P)[:],
                    in_=g2_sb[:])
            if STAGE >= 4:
                aggregate(G2[:], OUT, out_sb, False, b2_sb if has_b2 else None, cc2)
                nc.sync.dma_start(
                    out=out.rearrange("(w p) h -> p w h", p=P)[:], in_=out_sb[:])
    nc.compile()
    return nc


def kernel(x, edge_index, W1, b1, W2, b2):
    x = np.asarray(x, np.float32)
    edge_index = np.asarray(edge_index, np.int32)
    W1 = np.asarray(W1, np.float32)
    b1 = np.asarray(b1, np.float32)
    W2 = np.asarray(W2, np.float32)
    b2 = np.asarray(b2, np.float32)

    N, IN_C = x.shape
    HID = W1.shape[1]
    OUT = W2.shape[1]
    NPC = (N + M - 1) // M               # nodes per core
    NW = (NPC + P - 1) // P              # 128-dst windows per core
    NPAD = NW * P

    # ---- host: sharding / index prep (integer work only) ----
    E = edge_index.shape[1]
    loop = np.arange(N, dtype=np.int64)
    src = np.concatenate([edge_index[0].astype(np.int64), loop])
    dst = np.concatenate([edge_index[1].astype(np.int64), loop])
    degN = np.bincount(dst, minlength=N).astype(np.float32)

    tidx = (src // NPC) * NPAD + (src % NPC)      # row in all-gathered table
    core = dst // NPC
    dl_in_core = dst - core * NPC
    win = dl_in_core // P
    dl = dl_in_core % P
    grp = (tidx >= SPLIT).astype(np.int64)

    # sort edges by (core, window, group, tidx)
    order = np.lexsort((tidx, grp, win, core))
    core_s, win_s, grp_s, tidx_s, dl_s = (
        core[order], win[order], grp[order], tidx[order], dl[order])

    # counts per (core, window, group)
    key = (core_s * NW + win_s) * 2 + grp_s
    cnt = np.bincount(key, minlength=M * NW * 2).reshape(M, NW, 2)
    Ccap = np.ceil(cnt / P).astype(np.int64).max(axis=0)   # [NW, 2]
    CA = Ccap[:, 0].tolist()
    CB = Ccap[:, 1].tolist()

    colsA = max(8, int(np.sum(CA)) * 8)
    colsB = max(8, int(np.sum(CB)) * 8)
    TTOT = max(1, int(np.sum(CA) + np.sum(CB)))

    idxA_h = np.zeros((M, P, colsA), np.int16)
    idxB_h = np.zeros((M, P, colsB), np.int16)
    dloc_h = np.full((M, P, TTOT), -1.0, np.float32)

    bounds = np.concatenate([[0], np.cumsum(cnt.reshape(-1))])
    for c in range(M):
        offA = offB = tcol = 0
        for w in range(NW):
            for g in range(2):
                C = Ccap[w, g]
                if C == 0:
                    continue
                k = (c * NW + w) * 2 + g
                s, e = bounds[k], bounds[k + 1]
                n = e - s
                slots = C * P
                ia = np.zeros(slots, np.int64)
                ia[:n] = tidx_s[s:e] - g * SPLIT
                da = np.full(slots, -1.0, np.float32)
                da[:n] = dl_s[s:e]
                # idx wrapped: idx i -> [i%16, i//16], replicated over 8 groups
                blk = ia.reshape(C * 8, 16).T.astype(np.int16)
                tgt = (idxA_h, idxB_h)[g]
                off = offA if g == 0 else offB
                tgt[c, :, off:off + C * 8] = np.tile(blk, (8, 1))
                # dst_local: edge i -> [i%128, tile i//128]
                dloc_h[c, :, tcol:tcol + C] = da.reshape(C, P).T
                if g == 0:
                    offA += C * 8
                else:
                    offB += C * 8
                tcol += C

    has_b1 = bool(np.any(b1 != 0))
    has_b2 = bool(np.any(b2 != 0))

    nc = _build_nc(IN_C, HID, OUT, NW, CA, CB, has_b1, has_b2)

    iota_h = np.broadcast_to(
        np.arange(P, dtype=np.float32), (P, P)).copy()
    ident_h = np.eye(P, dtype=np.float32)
    b1_h = np.broadcast_to(b1, (P, HID)).copy()
    b2_h = np.broadcast_to(b2, (P, OUT)).copy()

    in_maps = []
    for c in range(M):
        xs = x[c * NPC:(c + 1) * NPC]
        xTp = np.zeros((IN_C, NPAD), np.float32)
        xTp[:, :xs.shape[0]] = xs.T
        dg = np.ones(NPAD, np.float32)
        dg[:min(NPC, N - c * NPC)] = degN[c * NPC:(c + 1) * NPC]
        in_maps.append({
            "xT": xTp, "W1": W1, "W2": W2, "b1": b1_h, "b2": b2_h,
            "deg": dg.reshape(NW, P).T.copy(),
            "idxA": idxA_h[c], "idxB": idxB_h[c], "dloc": dloc_h[c],
            "iota": iota_h, "ident": ident_h,
        })

    import time as _time
    t0 = _time.perf_counter()
    res = run_bass_kernel_spmd(nc, in_maps, list(range(M)))
    kernel.last_wall_s = _time.perf_counter() - t0

    out = np.concatenate(
        [res.results[c]["out"][:NPC] for c in range(M)], axis=0)[:N]
    return out.astype(np.float32)


# revision 15
# speedup vs baseline: 1.3007x; 1.3007x over previous
"""GCN encoder (2-layer) on 8 Trainium2 NeuronCores.

Sharding: nodes split into 8 contiguous ranges (graph/data parallel).
Each core:
  1. GEMM  g1 = dinv * (x_shard @ W1)          (dinv folded into table rows)
  2. AllGather g1 shards -> full table G1
  3. For each 128-dst window: dma_gather source rows of G1, build one-hot
     selection matrices S on DVE (iota == dst_local), scatter-add via
     PE matmul  psum += S.T @ rows ; h1 = relu(dinv * psum + b1)
  4. GEMM  g2 = dinv * (h1 @ W2), AllGather -> G2, same aggregation,
     out = dinv * psum + b2
Host side only does sharding/index prep (CSR-style sort, int16 index
packing) and the final unshard.
"""

import os
import numpy as np

import concourse.bass as bass
import concourse.bacc as bacc
import concourse.mybir as mybir
from concourse.tile import TileContext
from concourse.tile_rust import add_dep_helper
from concourse.bass_utils import run_bass_kernel_spmd

F32 = mybir.dt.float32
I16 = mybir.dt.int16
M = 8           # cores
P = 128         # partitions
SPLIT = 32768   # int16 index limit for dma_gather


def _build_nc(IN_C, HID, OUT, NW, CA, CB, has_b1, has_b2):
    """Build the SPMD Bass program. CA/CB: per-window tile capacities."""
    STAGE = float(os.environ.get("GCN_STAGE", "4"))
    NPAD = NW * P
    KT1 = IN_C // P
    colsA = max(8, int(np.sum(CA)) * 8)   # idx cols (16 idx per col)
    colsB = max(8, int(np.sum(CB)) * 8)
    TTOT = max(1, int(np.sum(CA) + np.sum(CB)))

    nc = bacc.Bacc("TRN2", target_bir_lowering=False, debug=False, num_devices=M)
    xT = nc.dram_tensor("xT", [IN_C, NPAD], F32, kind="ExternalInput")
    W1 = nc.dram_tensor("W1", [IN_C, HID], F32, kind="ExternalInput")
    W2 = nc.dram_tensor("W2", [HID, OUT], F32, kind="ExternalInput")
    b1 = nc.dram_tensor("b1", [P, HID], F32, kind="ExternalInput")
    b2 = nc.dram_tensor("b2", [P, OUT], F32, kind="ExternalInput")
    deg = nc.dram_tensor("deg", [P, NW], F32, kind="ExternalInput")
    idxA = nc.dram_tensor("idxA", [P, colsA], I16, kind="ExternalInput")
    idxB = nc.dram_tensor("idxB", [P, colsB], I16, kind="ExternalInput")
    dloc = nc.dram_tensor("dloc", [P, TTOT], F32, kind="ExternalInput")
    iota = nc.dram_tensor("iota", [P, 6 * P], F32, kind="ExternalInput")
    ident = nc.dram_tensor("ident", [P, P], F32, kind="ExternalInput")
    out = nc.dram_tensor("out", [NPAD, OUT], F32, kind="ExternalOutput")
    G1 = nc.dram_tensor("G1", [M * NPAD, HID], F32, addr_space="Shared")
    G2 = nc.dram_tensor("G2", [M * NPAD, OUT], F32, addr_space="Shared")

    xT_v = xT.rearrange("(kt k) n -> kt k n", k=P)
    W1_v = W1.rearrange("(kt k) h -> k kt h", k=P)

    with TileContext(nc) as tc:
        with (
            tc.tile_pool(name="const", bufs=1) as const,
            tc.tile_pool(name="big", bufs=1) as big,
            tc.tile_pool(name="work", bufs=4) as work,
            tc.tile_pool(name="spool", bufs=4) as spool,
            tc.tile_pool(name="gpool", bufs=2) as gpool,
            tc.tile_pool(name="psum", bufs=2, space="PSUM") as psum,
            tc.tile_pool(name="dram", bufs=1, space="DRAM") as dram,
        ):
            g1loc = dram.tile([NPAD, HID], F32)
            g2loc = dram.tile([NPAD, OUT], F32)
            # ---- constants ----
            w1_sb = const.tile([P, KT1, HID], F32)
            nc.sync.dma_start(out=w1_sb[:], in_=W1_v[:])
            w2_sb = const.tile([P, OUT], F32)
            nc.sync.dma_start(out=w2_sb[:], in_=W2[:])
            iota_sb = const.tile([P, 6 * P], F32)
            nc.sync.dma_start(out=iota_sb[:], in_=iota[:])
            id_sb = const.tile([P, P], F32)
            nc.sync.dma_start(out=id_sb[:], in_=ident[:])
            deg_sb = const.tile([P, NW], F32)
            nc.sync.dma_start(out=deg_sb[:], in_=deg[:])
            dloc_sb = big.tile([P, TTOT], F32)
            nc.sync.dma_start(out=dloc_sb[:], in_=dloc[:])
            b1_sb = const.tile([P, HID], F32)
            b2_sb = const.tile([P, OUT], F32)
            if has_b1:
                nc.sync.dma_start(out=b1_sb[:], in_=b1[:])
            if has_b2:
                nc.sync.dma_start(out=b2_sb[:], in_=b2[:])

            dinv_sb = const.tile([P, NW], F32)
            nc.vector.reciprocal(out=dinv_sb[:], in_=deg_sb[:])
            nc.scalar.sqrt(out=dinv_sb[:], in_=dinv_sb[:])

            g1_sb = big.tile([P, NW, HID], F32)
            h1_sb = big.tile([P, NW, HID], F32)
            g2_sb = big.tile([P, NW, OUT], F32)
            out_sb = big.tile([P, NW, OUT], F32)

            # ---- GEMM1: g1 = dinv * (x @ W1) ----
            for w in range(NW):
                ps = psum.tile([P, HID], F32, tag="g1ps")
                for k in range(KT1):
                    xt = work.tile([P, P], F32, tag="xt")
                    nc.sync.dma_start(
                        out=xt[:], in_=xT_v[k, :, w * P:(w + 1) * P])
                    nc.tensor.matmul(
                        ps[:], lhsT=xt[:], rhs=w1_sb[:, k, :],
                        start=(k == 0), stop=(k == KT1 - 1))
                nc.vector.tensor_scalar(
                    out=g1_sb[:, w, :], in0=ps[:],
                    scalar1=dinv_sb[:, w:w + 1], scalar2=None,
                    op0=mybir.AluOpType.mult)
            nc.gpsimd.dma_start(
                out=g1loc[:].rearrange("(w p) h -> p w h", p=P), in_=g1_sb[:])
            cc1 = nc.gpsimd.collective_compute(
                "AllGather", mybir.AluOpType.bypass,
                replica_groups=[list(range(M))],
                ins=[g1loc.opt()], outs=[G1[:]])

            if STAGE == 1:
                nc.sync.dma_start(
                    out=out.rearrange("(w p) h -> p w h", p=P)[:],
                    in_=g1_sb[:, :, :OUT])
            # ---- aggregation over edges ----
            def aggregate(G_ap, elem, dst_view, relu, bias_sb, cc):
                offA = offB = tcol = 0
                for w in range(NW):
                    ps = psum.tile([P, HID], F32, tag="agg")
                    nmm = CA[w] + CB[w]
                    mm = 0
                    CHUNK = 6   # <=768 idx per dma_gather (HW ring limit)
                    for g in range(2):
                        C = (CA, CB)[g][w]
                        if C == 0:
                            continue
                        off = (offA, offB)[g]
                        in_ap = G_ap[SPLIT:, :] if g == 1 else G_ap[:]
                        for c0 in range(0, C, CHUNK):
                            Cc = min(CHUNK, C - c0)
                            nidx = Cc * P
                            gb = gpool.tile([P, Cc, elem], F32, tag=f"gb{g}")
                            it = spool.tile([P, Cc * 8], I16, tag=f"idx{g}")
                            nc.sync.dma_start(
                                out=it[:],
                                in_=(idxA, idxB)[g][
                                    :, off + c0 * 8:off + (c0 + Cc) * 8])
                            gi = nc.gpsimd.dma_gather(
                                out_ap=gb[:], in_ap=in_ap,
                                idxs_ap=it[:],
                                num_idxs=nidx, num_idxs_reg=nidx,
                                elem_size=elem)
                            add_dep_helper(gi.ins, cc.ins, sync=True,
                                           reason="gather after allgather")
                            S = spool.tile([P, Cc, P], F32, tag="S")
                            nc.vector.tensor_tensor(
                                out=S[:],
                                in0=iota_sb[:, :Cc * P].rearrange(
                                    "p (c q) -> p c q", q=P),
                                in1=dloc_sb[:, tcol:tcol + Cc].to_broadcast(
                                    [P, Cc, P]),
                                op=mybir.AluOpType.is_equal)
                            for t in range(Cc):
                                nc.tensor.matmul(
                                    ps[:, :elem], lhsT=S[:, t, :],
                                    rhs=gb[:, t, :],
                                    start=(mm == 0), stop=(mm == nmm - 1))
                                mm += 1
                                tcol += 1
                        if g == 0:
                            offA += C * 8
                        else:
                            offB += C * 8
                    if bias_sb is None:
                        if relu:
                            nc.vector.tensor_scalar(
                                out=dst_view[:, w, :], in0=ps[:, :elem],
                                scalar1=dinv_sb[:, w:w + 1], scalar2=0.0,
                                op0=mybir.AluOpType.mult,
                                op1=mybir.AluOpType.max)
                        else:
                            nc.vector.tensor_scalar(
                                out=dst_view[:, w, :], in0=ps[:, :elem],
                                scalar1=dinv_sb[:, w:w + 1], scalar2=None,
                                op0=mybir.AluOpType.mult)
                    else:
                        tmp = work.tile([P, elem], F32, tag="evtmp")
                        nc.vector.tensor_scalar(
                            out=tmp[:], in0=ps[:, :elem],
                            scalar1=dinv_sb[:, w:w + 1], scalar2=None,
                            op0=mybir.AluOpType.mult)
                        if relu:
                            nc.vector.tensor_tensor(
                                out=tmp[:], in0=tmp[:], in1=bias_sb[:],
                                op=mybir.AluOpType.add)
                            nc.vector.tensor_scalar(
                                out=dst_view[:, w, :], in0=tmp[:],
                                scalar1=0.0, scalar2=None,
                                op0=mybir.AluOpType.max)
                        else:
                            nc.vector.tensor_tensor(
                                out=dst_view[:, w, :], in0=tmp[:],
                                in1=bias_sb[:], op=mybir.AluOpType.add)

            if STAGE == 1.5:
                tg = big.tile([P, NW, OUT], F32)
                nc.sync.dma_start(
                    out=tg[:], in_=G1.rearrange(
                        "(w p) h -> p w h", p=P)[:, :NW, :OUT])
                nc.sync.dma_start(
                    out=out.rearrange("(w p) h -> p w h", p=P)[:], in_=tg[:])
            if STAGE == 1.75:
                C0 = CA[0]
                gb0 = gpool.tile([P, C0, HID], F32, tag="gb0")
                it0 = spool.tile([P, C0 * 8], I16, tag="idx0")
                nc.sync.dma_start(out=it0[:], in_=idxA[:, 0:C0 * 8])
                nc.gpsimd.dma_gather(
                    out_ap=gb0[:], in_ap=G1[:],
                    idxs_ap=it0[:],
                    num_idxs=C0 * P, num_idxs_reg=C0 * P, elem_size=HID)
                nc.sync.dma_start(
                    out=out.rearrange("(w p) h -> p w h", p=P)[:, :1, :],
                    in_=gb0[:, :1, :OUT])
            if STAGE >= 2:
                aggregate(G1[:], HID, h1_sb, True, b1_sb if has_b1 else None, cc1)
            if STAGE == 2:
                nc.sync.dma_start(
                    out=out.rearrange("(w p) h -> p w h", p=P)[:],
                    in_=h1_sb[:, :, :OUT])

            # ---- GEMM2: g2 = dinv * (h1 @ W2) ----
            for w in (range(NW) if STAGE >= 3 else []):
                pst = psum.tile([P, P], F32, tag="tr")
                nc.tensor.transpose(
                    out=pst[:], in_=h1_sb[:, w, :], identity=id_sb[:])
                h1t = work.tile([P, P], F32, tag="h1t")
                nc.vector.tensor_copy(out=h1t[:], in_=pst[:])
                ps2 = psum.tile([P, OUT], F32, tag="mm2")
                nc.tensor.matmul(
                    ps2[:], lhsT=h1t[:], rhs=w2_sb[:], start=True, stop=True)
                nc.vector.tensor_scalar(
                    out=g2_sb[:, w, :], in0=ps2[:],
                    scalar1=dinv_sb[:, w:w + 1], scalar2=None,
                    op0=mybir.AluOpType.mult)
            if STAGE >= 3:
                nc.gpsimd.dma_start(
                    out=g2loc[:].rearrange("(w p) h -> p w h", p=P), in_=g2_sb[:])
                cc2 = nc.gpsimd.collective_compute(
                    "AllGather", mybir.AluOpType.bypass,
                    replica_groups=[list(range(M))],
                    ins=[g2loc.opt()], outs=[G2[:]])
            if STAGE == 3:
                nc.sync.dma_start(
                    out=out.rearrange("(w p) h -> p w h", p=P)[:],
                    in_=g2_sb[:])
            if STAGE >= 4:
                aggregate(G2[:], OUT, out_sb, False, b2_sb if has_b2 else None, cc2)
                nc.sync.dma_start(
                    out=out.rearrange("(w p) h -> p w h", p=P)[:], in_=out_sb[:])
    nc.compile()
    return nc


def kernel(x, edge_index, W1, b1, W2, b2):
    x = np.asarray(x, np.float32)
    edge_index = np.asarray(edge_index, np.int32)
    W1 = np.asarray(W1, np.float32)
    b1 = np.asarray(b1, np.float32)
    W2 = np.asarray(W2, np.float32)
    b2 = np.asarray(b2, np.float32)

    N, IN_C = x.shape
    HID = W1.shape[1]
    OUT = W2.shape[1]
    NPC = (N + M - 1) // M               # nodes per core
    NW = (NPC + P - 1) // P              # 128-dst windows per core
    NPAD = NW * P

    # ---- host: sharding / index prep (integer work only) ----
    E = edge_index.shape[1]
    loop = np.arange(N, dtype=np.int64)
    src = np.concatenate([edge_index[0].astype(np.int64), loop])
    dst = np.concatenate([edge_index[1].astype(np.int64), loop])
    degN = np.bincount(dst, minlength=N).astype(np.float32)

    tidx = (src // NPC) * NPAD + (src % NPC)      # row in all-gathered table
    core = dst // NPC
    dl_in_core = dst - core * NPC
    win = dl_in_core // P
    dl = dl_in_core % P
    grp = (tidx >= SPLIT).astype(np.int64)

    # sort edges by (core, window, group, tidx)
    order = np.lexsort((tidx, grp, win, core))
    core_s, win_s, grp_s, tidx_s, dl_s = (
        core[order], win[order], grp[order], tidx[order], dl[order])

    # counts per (core, window, group)
    key = (core_s * NW + win_s) * 2 + grp_s
    cnt = np.bincount(key, minlength=M * NW * 2).reshape(M, NW, 2)
    Ccap = np.ceil(cnt / P).astype(np.int64).max(axis=0)   # [NW, 2]
    CA = Ccap[:, 0].tolist()
    CB = Ccap[:, 1].tolist()

    colsA = max(8, int(np.sum(CA)) * 8)
    colsB = max(8, int(np.sum(CB)) * 8)
    TTOT = max(1, int(np.sum(CA) + np.sum(CB)))

    idxA_h = np.zeros((M, P, colsA), np.int16)
    idxB_h = np.zeros((M, P, colsB), np.int16)
    dloc_h = np.full((M, P, TTOT), -1.0, np.float32)

    bounds = np.concatenate([[0], np.cumsum(cnt.reshape(-1))])
    for c in range(M):
        offA = offB = tcol = 0
        for w in range(NW):
            for g in range(2):
                C = Ccap[w, g]
                if C == 0:
                    continue
                k = (c * NW + w) * 2 + g
                s, e = bounds[k], bounds[k + 1]
                n = e - s
                slots = C * P
                ia = np.zeros(slots, np.int64)
                ia[:n] = tidx_s[s:e] - g * SPLIT
                da = np.full(slots, -1.0, np.float32)
                da[:n] = dl_s[s:e]
                # idx wrapped: idx i -> [i%16, i//16], replicated over 8 groups
                blk = ia.reshape(C * 8, 16).T.astype(np.int16)
                tgt = (idxA_h, idxB_h)[g]
                off = offA if g == 0 else offB
                tgt[c, :, off:off + C * 8] = np.tile(blk, (8, 1))
                # dst_local: edge i -> [i%128, tile i//128]
                dloc_h[c, :, tcol:tcol + C] = da.reshape(C, P).T
                if g == 0:
                    offA += C * 8
                else:
                    offB += C * 8
                tcol += C

    has_b1 = bool(np.any(b1 != 0))
    has_b2 = bool(np.any(b2 != 0))

    nc = _build_nc(IN_C, HID, OUT, NW, CA, CB, has_b1, has_b2)

    iota_h = np.broadcast_to(
        np.tile(np.arange(P, dtype=np.float32), 6), (P, 6 * P)).copy()
    ident_h = np.eye(P, dtype=np.float32)
    b1_h = np.broadcast_to(b1, (P, HID)).copy()
    b2_h = np.broadcast_to(b2, (P, OUT)).copy()

    in_maps = []
    for c in range(M):
        xs = x[c * NPC:(c + 1) * NPC]
        xTp = np.zeros((IN_C, NPAD), np.float32)
        xTp[:, :xs.shape[0]] = xs.T
        dg = np.ones(NPAD, np.float32)
        dg[:min(NPC, N - c * NPC)] = degN[c * NPC:(c + 1) * NPC]
        in_maps.append({
            "xT": xTp, "W1": W1, "W2": W2, "b1": b1_h, "b2": b2_h,
            "deg": dg.reshape(NW, P).T.copy(),
            "idxA": idxA_h[c], "idxB": idxB_h[c], "dloc": dloc_h[c],
            "iota": iota_h, "ident": ident_h,
        })

    import time as _time
    t0 = _time.perf_counter()
    res = run_bass_kernel_spmd(nc, in_maps, list(range(M)))
    kernel.last_wall_s = _time.perf_counter() - t0
    if os.environ.get("GCN_BENCH"):
        times = []
        for _ in range(int(os.environ["GCN_BENCH"])):
            t0 = _time.perf_counter()
            res = run_bass_kernel_spmd(nc, in_maps, list(range(M)))
            times.append(_time.perf_counter() - t0)
        kernel.bench_times = times
        print("bench times:", [f"{t:.3f}" for t in times], flush=True)

    out = np.concatenate(
        [res.results[c]["out"][:NPC] for c in range(M)], axis=0)[:N]
    return out.astype(np.float32)


# revision 16
# speedup vs baseline: 1.7092x; 1.3141x over previous
"""GCN encoder (2-layer) on 8 Trainium2 NeuronCores.

Sharding: nodes split into 8 contiguous ranges (graph/data parallel).
Each core:
  1. GEMM  g1 = dinv * (x_shard @ W1)          (dinv folded into table rows)
  2. AllGather g1 shards -> full table G1
  3. For each 128-dst window: dma_gather source rows of G1, build one-hot
     selection matrices S on DVE (iota == dst_local), scatter-add via
     PE matmul  psum += S.T @ rows ; h1 = relu(dinv * psum + b1)
  4. GEMM  g2 = dinv * (h1 @ W2), AllGather -> G2, same aggregation,
     out = dinv * psum + b2
Host side only does sharding/index prep (CSR-style sort, int16 index
packing) and the final unshard.
"""

import os
import numpy as np

import concourse.bass as bass
import concourse.bacc as bacc
import concourse.mybir as mybir
from concourse.tile import TileContext
from concourse.tile_rust import add_dep_helper
from concourse.bass_utils import run_bass_kernel_spmd

F32 = mybir.dt.float32
I16 = mybir.dt.int16
M = 8           # cores
P = 128         # partitions
SPLIT = 32768   # int16 index limit for dma_gather


def _build_nc(IN_C, HID, OUT, NW, CA, CB, has_b1, has_b2):
    """Build the SPMD Bass program. CA/CB: per-window tile capacities."""
    NPAD = NW * P
    KT1 = IN_C // P
    colsA = max(8, int(np.sum(CA)) * 8)   # idx cols (16 idx per col)
    colsB = max(8, int(np.sum(CB)) * 8)
    TTOT = max(1, int(np.sum(CA) + np.sum(CB)))

    nc = bacc.Bacc("TRN2", target_bir_lowering=False, debug=False, num_devices=M)
    xT = nc.dram_tensor("xT", [IN_C, NPAD], F32, kind="ExternalInput")
    W1 = nc.dram_tensor("W1", [IN_C, HID], F32, kind="ExternalInput")
    W2 = nc.dram_tensor("W2", [HID, OUT], F32, kind="ExternalInput")
    b1 = nc.dram_tensor("b1", [P, HID], F32, kind="ExternalInput")
    b2 = nc.dram_tensor("b2", [P, OUT], F32, kind="ExternalInput")
    deg = nc.dram_tensor("deg", [P, NW], F32, kind="ExternalInput")
    idxA = nc.dram_tensor("idxA", [P, colsA], I16, kind="ExternalInput")
    idxB = nc.dram_tensor("idxB", [P, colsB], I16, kind="ExternalInput")
    dloc = nc.dram_tensor("dloc", [P, TTOT], F32, kind="ExternalInput")
    iota = nc.dram_tensor("iota", [P, 6 * P], F32, kind="ExternalInput")
    ident = nc.dram_tensor("ident", [P, P], F32, kind="ExternalInput")
    out = nc.dram_tensor("out", [NPAD, OUT], F32, kind="ExternalOutput")
    G1 = nc.dram_tensor("G1", [M * NPAD, HID], F32, addr_space="Shared")
    G2 = nc.dram_tensor("G2", [M * NPAD, OUT], F32, addr_space="Shared")

    xT_v = xT.rearrange("(kt k) n -> kt k n", k=P)
    W1_v = W1.rearrange("(kt k) h -> k kt h", k=P)

    with TileContext(nc) as tc:
        with (
            tc.tile_pool(name="const", bufs=1) as const,
            tc.tile_pool(name="big", bufs=1) as big,
            tc.tile_pool(name="work", bufs=4) as work,
            tc.tile_pool(name="spool", bufs=4) as spool,
            tc.tile_pool(name="gpool", bufs=3) as gpool,
            tc.tile_pool(name="psum", bufs=2, space="PSUM") as psum,
            tc.tile_pool(name="dram", bufs=1, space="DRAM") as dram,
        ):
            g1loc = dram.tile([NPAD, HID], F32)
            g2loc = dram.tile([NPAD, OUT], F32)
            # ---- constants ----
            w1_sb = const.tile([P, KT1, HID], F32)
            nc.sync.dma_start(out=w1_sb[:], in_=W1_v[:])
            w2_sb = const.tile([P, OUT], F32)
            nc.sync.dma_start(out=w2_sb[:], in_=W2[:])
            iota_sb = const.tile([P, 6 * P], F32)
            nc.sync.dma_start(out=iota_sb[:], in_=iota[:])
            id_sb = const.tile([P, P], F32)
            nc.sync.dma_start(out=id_sb[:], in_=ident[:])
            deg_sb = const.tile([P, NW], F32)
            nc.sync.dma_start(out=deg_sb[:], in_=deg[:])
            dloc_sb = big.tile([P, TTOT], F32)
            nc.sync.dma_start(out=dloc_sb[:], in_=dloc[:])
            b1_sb = const.tile([P, HID], F32)
            b2_sb = const.tile([P, OUT], F32)
            if has_b1:
                nc.sync.dma_start(out=b1_sb[:], in_=b1[:])
            if has_b2:
                nc.sync.dma_start(out=b2_sb[:], in_=b2[:])

            dinv_sb = const.tile([P, NW], F32)
            nc.vector.reciprocal(out=dinv_sb[:], in_=deg_sb[:])
            nc.scalar.sqrt(out=dinv_sb[:], in_=dinv_sb[:])

            g1_sb = big.tile([P, NW, HID], F32)
            h1_sb = big.tile([P, NW, HID], F32)
            g2_sb = big.tile([P, NW, OUT], F32)
            out_sb = big.tile([P, NW, OUT], F32)

            # ---- GEMM1: g1 = dinv * (x @ W1) ----
            for w in range(NW):
                ps = psum.tile([P, HID], F32, tag="g1ps")
                for k in range(KT1):
                    xt = work.tile([P, P], F32, tag="xt")
                    nc.sync.dma_start(
                        out=xt[:], in_=xT_v[k, :, w * P:(w + 1) * P])
                    nc.tensor.matmul(
                        ps[:], lhsT=xt[:], rhs=w1_sb[:, k, :],
                        start=(k == 0), stop=(k == KT1 - 1))
                nc.vector.tensor_scalar(
                    out=g1_sb[:, w, :], in0=ps[:],
                    scalar1=dinv_sb[:, w:w + 1], scalar2=None,
                    op0=mybir.AluOpType.mult)
            nc.gpsimd.dma_start(
                out=g1loc[:].rearrange("(w p) h -> p w h", p=P), in_=g1_sb[:])
            cc1 = nc.gpsimd.collective_compute(
                "AllGather", mybir.AluOpType.bypass,
                replica_groups=[list(range(M))],
                ins=[g1loc.opt()], outs=[G1[:]])

            # ---- aggregation over edges ----
            def aggregate(G_ap, elem, dst_view, relu, bias_sb, cc):
                offA = offB = tcol = 0
                for w in range(NW):
                    ps = psum.tile([P, HID], F32, tag="agg")
                    nmm = CA[w] + CB[w]
                    mm = 0
                    CHUNK = 6   # <=768 idx per dma_gather (HW ring limit)
                    for g in range(2):
                        C = (CA, CB)[g][w]
                        if C == 0:
                            continue
                        off = (offA, offB)[g]
                        in_ap = G_ap[SPLIT:, :] if g == 1 else G_ap[:]
                        for c0 in range(0, C, CHUNK):
                            Cc = min(CHUNK, C - c0)
                            nidx = Cc * P
                            gb = gpool.tile([P, Cc, elem], F32, tag=f"gb{g}")
                            it = spool.tile([P, Cc * 8], I16, tag=f"idx{g}")
                            nc.sync.dma_start(
                                out=it[:],
                                in_=(idxA, idxB)[g][
                                    :, off + c0 * 8:off + (c0 + Cc) * 8])
                            gi = nc.gpsimd.dma_gather(
                                out_ap=gb[:], in_ap=in_ap,
                                idxs_ap=it[:],
                                num_idxs=nidx, num_idxs_reg=nidx,
                                elem_size=elem)
                            add_dep_helper(gi.ins, cc.ins, sync=True,
                                           reason="gather after allgather")
                            S = spool.tile([P, Cc, P], F32, tag="S")
                            nc.vector.tensor_tensor(
                                out=S[:],
                                in0=iota_sb[:, :Cc * P].rearrange(
                                    "p (c q) -> p c q", q=P),
                                in1=dloc_sb[:, tcol:tcol + Cc].to_broadcast(
                                    [P, Cc, P]),
                                op=mybir.AluOpType.is_equal)
                            for t in range(Cc):
                                nc.tensor.matmul(
                                    ps[:, :elem], lhsT=S[:, t, :],
                                    rhs=gb[:, t, :],
                                    start=(mm == 0), stop=(mm == nmm - 1))
                                mm += 1
                                tcol += 1
                        if g == 0:
                            offA += C * 8
                        else:
                            offB += C * 8
                    if bias_sb is None:
                        if relu:
                            nc.vector.tensor_scalar(
                                out=dst_view[:, w, :], in0=ps[:, :elem],
                                scalar1=dinv_sb[:, w:w + 1], scalar2=0.0,
                                op0=mybir.AluOpType.mult,
                                op1=mybir.AluOpType.max)
                        else:
                            nc.vector.tensor_scalar(
                                out=dst_view[:, w, :], in0=ps[:, :elem],
                                scalar1=dinv_sb[:, w:w + 1], scalar2=None,
                                op0=mybir.AluOpType.mult)
                    else:
                        tmp = work.tile([P, elem], F32, tag="evtmp")
                        nc.vector.tensor_scalar(
                            out=tmp[:], in0=ps[:, :elem],
                            scalar1=dinv_sb[:, w:w + 1], scalar2=None,
                            op0=mybir.AluOpType.mult)
                        if relu:
                            nc.vector.tensor_tensor(
                                out=tmp[:], in0=tmp[:], in1=bias_sb[:],
                                op=mybir.AluOpType.add)
                            nc.vector.tensor_scalar(
                                out=dst_view[:, w, :], in0=tmp[:],
                                scalar1=0.0, scalar2=None,
                                op0=mybir.AluOpType.max)
                        else:
                            nc.vector.tensor_tensor(
                                out=dst_view[:, w, :], in0=tmp[:],
                                in1=bias_sb[:], op=mybir.AluOpType.add)

            aggregate(G1[:], HID, h1_sb, True, b1_sb if has_b1 else None, cc1)

            # ---- GEMM2: g2 = dinv * (h1 @ W2) ----
            for w in range(NW):
                pst = psum.tile([P, P], F32, tag="tr")
                nc.tensor.transpose(
                    out=pst[:], in_=h1_sb[:, w, :], identity=id_sb[:])
                h1t = work.tile([P, P], F32, tag="h1t")
                nc.vector.tensor_copy(out=h1t[:], in_=pst[:])
                ps2 = psum.tile([P, OUT], F32, tag="mm2")
                nc.tensor.matmul(
                    ps2[:], lhsT=h1t[:], rhs=w2_sb[:], start=True, stop=True)
                nc.vector.tensor_scalar(
                    out=g2_sb[:, w, :], in0=ps2[:],
                    scalar1=dinv_sb[:, w:w + 1], scalar2=None,
                    op0=mybir.AluOpType.mult)
            nc.gpsimd.dma_start(
                out=g2loc[:].rearrange("(w p) h -> p w h", p=P), in_=g2_sb[:])
            cc2 = nc.gpsimd.collective_compute(
                "AllGather", mybir.AluOpType.bypass,
                replica_groups=[list(range(M))],
                ins=[g2loc.opt()], outs=[G2[:]])

            aggregate(G2[:], OUT, out_sb, False, b2_sb if has_b2 else None, cc2)
            nc.sync.dma_start(
                out=out.rearrange("(w p) h -> p w h", p=P)[:], in_=out_sb[:])
    nc.compile()
    return nc


def kernel(x, edge_index, W1, b1, W2, b2):
    x = np.asarray(x, np.float32)
    edge_index = np.asarray(edge_index, np.int32)
    W1 = np.asarray(W1, np.float32)
    b1 = np.asarray(b1, np.float32)
    W2 = np.asarray(W2, np.float32)
    b2 = np.asarray(b2, np.float32)

    N, IN_C = x.shape
    HID = W1.shape[1]
    OUT = W2.shape[1]
    NPC = (N + M - 1) // M               # nodes per core
    NW = (NPC + P - 1) // P              # 128-dst windows per core
    NPAD = NW * P

    # ---- host: sharding / index prep (integer work only) ----
    E = edge_index.shape[1]
    loop = np.arange(N, dtype=np.int64)
    src = np.concatenate([edge_index[0].astype(np.int64), loop])
    dst = np.concatenate([edge_index[1].astype(np.int64), loop])
    degN = np.bincount(dst, minlength=N).astype(np.float32)

    tidx = (src // NPC) * NPAD + (src % NPC)      # row in all-gathered table
    core = dst // NPC
    dl_in_core = dst - core * NPC
    win = dl_in_core // P
    dl = dl_in_core % P
    grp = (tidx >= SPLIT).astype(np.int64)

    # sort edges by (core, window, group, tidx)
    order = np.lexsort((tidx, grp, win, core))
    core_s, win_s, grp_s, tidx_s, dl_s = (
        core[order], win[order], grp[order], tidx[order], dl[order])

    # counts per (core, window, group)
    key = (core_s * NW + win_s) * 2 + grp_s
    cnt = np.bincount(key, minlength=M * NW * 2).reshape(M, NW, 2)
    Ccap = np.ceil(cnt / P).astype(np.int64).max(axis=0)   # [NW, 2]
    CA = Ccap[:, 0].tolist()
    CB = Ccap[:, 1].tolist()

    colsA = max(8, int(np.sum(CA)) * 8)
    colsB = max(8, int(np.sum(CB)) * 8)
    TTOT = max(1, int(np.sum(CA) + np.sum(CB)))

    idxA_h = np.zeros((M, P, colsA), np.int16)
    idxB_h = np.zeros((M, P, colsB), np.int16)
    dloc_h = np.full((M, P, TTOT), -1.0, np.float32)

    bounds = np.concatenate([[0], np.cumsum(cnt.reshape(-1))])
    for c in range(M):
        offA = offB = tcol = 0
        for w in range(NW):
            for g in range(2):
                C = Ccap[w, g]
                if C == 0:
                    continue
                k = (c * NW + w) * 2 + g
                s, e = bounds[k], bounds[k + 1]
                n = e - s
                slots = C * P
                ia = np.zeros(slots, np.int64)
                ia[:n] = tidx_s[s:e] - g * SPLIT
                da = np.full(slots, -1.0, np.float32)
                da[:n] = dl_s[s:e]
                # idx wrapped: idx i -> [i%16, i//16], replicated over 8 groups
                blk = ia.reshape(C * 8, 16).T.astype(np.int16)
                tgt = (idxA_h, idxB_h)[g]
                off = offA if g == 0 else offB
                tgt[c, :, off:off + C * 8] = np.tile(blk, (8, 1))
                # dst_local: edge i -> [i%128, tile i//128]
                dloc_h[c, :, tcol:tcol + C] = da.reshape(C, P).T
                if g == 0:
                    offA += C * 8
                else:
                    offB += C * 8
                tcol += C

    has_b1 = bool(np.any(b1 != 0))
    has_b2 = bool(np.any(b2 != 0))

    nc = _build_nc(IN_C, HID, OUT, NW, CA, CB, has_b1, has_b2)

    iota_h = np.broadcast_to(
        np.tile(np.arange(P, dtype=np.float32), 6), (P, 6 * P)).copy()
    ident_h = np.eye(P, dtype=np.float32)
    b1_h = np.broadcast_to(b1, (P, HID)).copy()
    b2_h = np.broadcast_to(b2, (P, OUT)).copy()

    in_maps = []
    for c in range(M):
        xs = x[c * NPC:(c + 1) * NPC]
        xTp = np.zeros((IN_C, NPAD), np.float32)
        xTp[:, :xs.shape[0]] = xs.T
        dg = np.ones(NPAD, np.float32)
        dg[:min(NPC, N - c * NPC)] = degN[c * NPC:(c + 1) * NPC]
        in_maps.append({
            "xT": xTp, "W1": W1, "W2": W2, "b1": b1_h, "b2": b2_h,
            "deg": dg.reshape(NW, P).T.copy(),
            "idxA": idxA_h[c], "idxB": idxB_h[c], "dloc": dloc_h[c],
            "iota": iota_h, "ident": ident_h,
        })

    import time as _time
    t0 = _time.perf_counter()
    res = run_bass_kernel_spmd(nc, in_maps, list(range(M)))
    kernel.last_wall_s = _time.perf_counter() - t0
    if os.environ.get("GCN_BENCH"):
        times = []
        for _ in range(int(os.environ["GCN_BENCH"])):
            t0 = _time.perf_counter()
            res = run_bass_kernel_spmd(nc, in_maps, list(range(M)))
            times.append(_time.perf_counter() - t0)
        kernel.bench_times = times
        print("bench times:", [f"{t:.3f}" for t in times], flush=True)

    out = np.concatenate(
        [res.results[c]["out"][:NPC] for c in range(M)], axis=0)[:N]
    return out.astype(np.float32)
